# revision 4
# baseline (speedup 1.0000x reference)
"""Trainium2 Bass kernel: GNN message passing  out = relu((adj @ x) @ W.T + b).

Sharding: 1D row partition of adj across 8 NeuronCores (1024 rows each).
Each core computes aggT = x.T @ adjT_c (= (adj_c @ x).T) with x-tiles as the
stationary operand and the pre-transposed adj shard streaming as the moving
operand, accumulating f32 in PSUM over the 8192-deep contraction.  The second
linear runs as outT = (W.T-tiles) @ aggT so the bias lands on the partition
dim, letting the scalar engine fuse bias+ReLU while reading PSUM.  Host-side
numpy does the adj transpose + bf16 casts and re-assembles the full output.
"""

import numpy as np
import ml_dtypes

import concourse.mybir as mybir
from concourse import bacc
from concourse.tile import TileContext
from concourse.bass_utils import run_bass_kernel_spmd

P = 128
N_NODES = 8192
DIM = 512
NCORES = 8
M = N_NODES // NCORES          # 1024 output rows per core
KT = N_NODES // P              # 64 contraction tiles
NT = DIM // P                  # 4 tiles of the hidden dim (MM1 output part.)
JT = DIM // P                  # 4 tiles of the output-feature dim
FREE = 512                     # moving free dim / PSUM bank width (f32)
MCH = M // FREE                # 2 moving chunks per adj tile row block
BF16 = mybir.dt.bfloat16
F32 = mybir.dt.float32

_NC = None


def _build_nc():
    nc = bacc.Bacc("TRN2", debug=False)
    x_d = nc.dram_tensor("x", [N_NODES, DIM], BF16, kind="ExternalInput").ap()
    adjt_d = nc.dram_tensor("adjt", [N_NODES, M], BF16, kind="ExternalInput").ap()
    wt_d = nc.dram_tensor("wt", [DIM, DIM], BF16, kind="ExternalInput").ap()
    b_d = nc.dram_tensor("b", [P, JT], F32, kind="ExternalInput").ap()
    out_d = nc.dram_tensor("outt", [DIM, M], F32, kind="ExternalOutput").ap()

    with TileContext(nc) as tc:
        with (
            tc.tile_pool(name="xsb", bufs=1) as xpool,
            tc.tile_pool(name="wsb", bufs=1) as wpool,
            tc.tile_pool(name="adj", bufs=6) as adjpool,
            tc.tile_pool(name="agg", bufs=1) as aggpool,
            tc.tile_pool(name="osb", bufs=4) as opool,
            tc.tile_pool(name="ps", bufs=8, space="PSUM") as pspool,
        ):
            # Resident stationary operands: x (64 KB/part) and W.T (4 KB/part)
            x_sb = xpool.tile([P, KT * DIM], BF16)
            for k in range(KT):
                nc.sync.dma_start(
                    x_sb[:, k * DIM : (k + 1) * DIM], x_d[k * P : (k + 1) * P, :]
                )
            wt_sb = wpool.tile([P, NT * DIM], BF16)
            for n in range(NT):
                nc.sync.dma_start(
                    wt_sb[:, n * DIM : (n + 1) * DIM], wt_d[n * P : (n + 1) * P, :]
                )
            b_sb = wpool.tile([P, JT], F32)
            nc.sync.dma_start(b_sb[:], b_d[:])

            # MM1: aggT[n*128+a, mc*512+m] accumulated in 8 PSUM banks.
            agg_ps = [
                [
                    pspool.tile([P, FREE], F32, tag="ps", name=f"aggps_{n}_{mc}")
                    for mc in range(MCH)
                ]
                for n in range(NT)
            ]
            for k in range(KT):
                adj_sb = adjpool.tile([P, M], BF16, tag="adj", name=f"adj_{k}")
                nc.sync.dma_start(adj_sb[:], adjt_d[k * P : (k + 1) * P, :])
                for n in range(NT):
                    for mc in range(MCH):
                        nc.tensor.matmul(
                            agg_ps[n][mc][:],
                            x_sb[:, k * DIM + n * P : k * DIM + (n + 1) * P],
                            adj_sb[:, mc * FREE : (mc + 1) * FREE],
                            start=(k == 0),
                            stop=(k == KT - 1),
                        )

            # PSUM -> SBUF (cast to bf16) so MM2 can stream aggT as moving op.
            agg_sb = aggpool.tile([P, NT * M], BF16)
            for n in range(NT):
                for mc in range(MCH):
                    nc.vector.tensor_copy(
                        agg_sb[:, n * M + mc * FREE : n * M + (mc + 1) * FREE],
                        agg_ps[n][mc][:],
                    )

            # MM2 + fused bias/ReLU on the scalar engine, then store.
            for mc in range(MCH):
                for j in range(JT):
                    out_ps = pspool.tile([P, FREE], F32, tag="ps", name=f"ops_{mc}_{j}")
                    for n in range(NT):
                        nc.tensor.matmul(
                            out_ps[:],
                            wt_sb[:, n * DIM + j * P : n * DIM + (j + 1) * P],
                            agg_sb[:, n * M + mc * FREE : n * M + (mc + 1) * FREE],
                            start=(n == 0),
                            stop=(n == NT - 1),
                        )
                    out_sb = opool.tile([P, FREE], F32, tag="osb", name=f"o_{mc}_{j}")
                    nc.scalar.activation(
                        out_sb[:],
                        out_ps[:],
                        mybir.ActivationFunctionType.Relu,
                        bias=b_sb[:, j : j + 1],
                    )
                    nc.sync.dma_start(
                        out_d[j * P : (j + 1) * P, mc * FREE : (mc + 1) * FREE],
                        out_sb[:],
                    )
    nc.finalize()
    return nc


def _get_nc():
    global _NC
    if _NC is None:
        _NC = _build_nc()
    return _NC


def _prepare(inputs):
    bf = ml_dtypes.bfloat16
    x = np.asarray(inputs["x"], dtype=np.float32)
    adj = np.asarray(inputs["adj"], dtype=np.float32)
    W = np.asarray(inputs["W"], dtype=np.float32)
    b = np.asarray(inputs["b"], dtype=np.float32)

    x_bf = np.ascontiguousarray(x.astype(bf))
    wt_bf = np.ascontiguousarray(W.T.astype(bf))
    adjt_bf = adj.astype(bf).T  # [K, rows] view
    b_tiled = np.ascontiguousarray(b.reshape(JT, P).T)  # [128, 4]

    in_maps = []
    for c in range(NCORES):
        in_maps.append(
            {
                "x": x_bf,
                "adjt": np.ascontiguousarray(adjt_bf[:, c * M : (c + 1) * M]),
                "wt": wt_bf,
                "b": b_tiled,
            }
        )
    return in_maps


def _run(in_maps, **kwargs):
    return run_bass_kernel_spmd(
        _get_nc(), in_maps, core_ids=list(range(NCORES)), **kwargs
    )


def _assemble(results):
    out = np.empty((N_NODES, DIM), dtype=np.float32)
    for c in range(NCORES):
        out[c * M : (c + 1) * M, :] = results[c]["outt"].T
    return out


def kernel(**inputs):
    res = _run(_prepare(inputs))
    return _assemble(res.results)


# revision 7
# speedup vs baseline: 1.2871x; 1.2871x over previous
"""Trainium2 Bass kernel: GNN message passing  out = relu((adj @ x) @ W.T + b).

Sharding: 1D row partition of adj across 8 NeuronCores (1024 rows each).
Each core computes aggT = x.T @ adjT_c (= (adj_c @ x).T) with x-tiles as the
stationary operand and the pre-transposed adj shard streaming as the moving
operand, accumulating f32 in PSUM over the 8192-deep contraction.  The second
linear runs as outT = (W.T-tiles) @ aggT so the bias lands on the partition
dim, letting the scalar engine fuse bias+ReLU while reading PSUM.  Host-side
numpy does the adj transpose + bf16 casts and re-assembles the full output.
"""

import numpy as np
import ml_dtypes

import concourse.mybir as mybir
from concourse import bacc
from concourse.tile import TileContext
from concourse.bass_utils import run_bass_kernel_spmd

P = 128
N_NODES = 8192
DIM = 512
NCORES = 8
M = N_NODES // NCORES          # 1024 output rows per core
KT = N_NODES // P              # 64 contraction tiles
NT = DIM // P                  # 4 tiles of the hidden dim (MM1 output part.)
JT = DIM // P                  # 4 tiles of the output-feature dim
FREE = 512                     # moving free dim / PSUM bank width (f32)
MCH = M // FREE                # 2 moving chunks per adj tile row block
BF16 = mybir.dt.bfloat16
F32 = mybir.dt.float32

_NC = None


def _build_nc():
    nc = bacc.Bacc("TRN2", debug=False)
    x_d = nc.dram_tensor("x", [N_NODES, DIM], BF16, kind="ExternalInput").ap()
    adjt_d = nc.dram_tensor("adjt", [N_NODES, M], BF16, kind="ExternalInput").ap()
    wt_d = nc.dram_tensor("wt", [DIM, DIM], BF16, kind="ExternalInput").ap()
    b_d = nc.dram_tensor("b", [P, JT], F32, kind="ExternalInput").ap()
    out_d = nc.dram_tensor("outt", [DIM, M], F32, kind="ExternalOutput").ap()

    with TileContext(nc) as tc:
        with (
            tc.tile_pool(name="xsb", bufs=1) as xpool,
            tc.tile_pool(name="wsb", bufs=1) as wpool,
            tc.tile_pool(name="adj", bufs=10) as adjpool,
            tc.tile_pool(name="agg", bufs=1) as aggpool,
            tc.tile_pool(name="osb", bufs=4) as opool,
            tc.tile_pool(name="ps", bufs=8, space="PSUM") as pspool,
        ):
            # Resident stationary operands: x (64 KB/part) and W.T (4 KB/part).
            # x tile loads are interleaved into the k loop below so the 8 MiB
            # x preload doesn't starve the adj stream.
            x_sb = xpool.tile([P, KT * DIM], BF16)
            wt_sb = wpool.tile([P, NT * DIM], BF16)
            for n in range(NT):
                nc.sync.dma_start(
                    wt_sb[:, n * DIM : (n + 1) * DIM], wt_d[n * P : (n + 1) * P, :]
                )
            b_sb = wpool.tile([P, JT], F32)
            nc.sync.dma_start(b_sb[:], b_d[:])

            # MM1: aggT[n*128+a, mc*512+m] accumulated in 8 PSUM banks.
            agg_ps = [
                [
                    pspool.tile([P, FREE], F32, tag="ps", name=f"aggps_{n}_{mc}")
                    for mc in range(MCH)
                ]
                for n in range(NT)
            ]
            for k in range(KT):
                nc.sync.dma_start(
                    x_sb[:, k * DIM : (k + 1) * DIM], x_d[k * P : (k + 1) * P, :]
                )
                adj_sb = adjpool.tile([P, M], BF16, tag="adj", name=f"adj_{k}")
                nc.sync.dma_start(adj_sb[:], adjt_d[k * P : (k + 1) * P, :])
                for n in range(NT):
                    for mc in range(MCH):
                        nc.tensor.matmul(
                            agg_ps[n][mc][:],
                            x_sb[:, k * DIM + n * P : k * DIM + (n + 1) * P],
                            adj_sb[:, mc * FREE : (mc + 1) * FREE],
                            start=(k == 0),
                            stop=(k == KT - 1),
                        )

            # PSUM -> SBUF (cast to bf16) so MM2 can stream aggT as moving op.
            agg_sb = aggpool.tile([P, NT * M], BF16)
            for n in range(NT):
                for mc in range(MCH):
                    nc.vector.tensor_copy(
                        agg_sb[:, n * M + mc * FREE : n * M + (mc + 1) * FREE],
                        agg_ps[n][mc][:],
                    )

            # MM2 + fused bias/ReLU on the scalar engine, then store.
            for mc in range(MCH):
                for j in range(JT):
                    out_ps = pspool.tile([P, FREE], F32, tag="ps", name=f"ops_{mc}_{j}")
                    for n in range(NT):
                        nc.tensor.matmul(
                            out_ps[:],
                            wt_sb[:, n * DIM + j * P : n * DIM + (j + 1) * P],
                            agg_sb[:, n * M + mc * FREE : n * M + (mc + 1) * FREE],
                            start=(n == 0),
                            stop=(n == NT - 1),
                        )
                    out_sb = opool.tile([P, FREE], F32, tag="osb", name=f"o_{mc}_{j}")
                    nc.scalar.activation(
                        out_sb[:],
                        out_ps[:],
                        mybir.ActivationFunctionType.Relu,
                        bias=b_sb[:, j : j + 1],
                    )
                    nc.sync.dma_start(
                        out_d[j * P : (j + 1) * P, mc * FREE : (mc + 1) * FREE],
                        out_sb[:],
                    )
    nc.finalize()
    return nc


def _get_nc():
    global _NC
    if _NC is None:
        _NC = _build_nc()
    return _NC


def _prepare(inputs):
    bf = ml_dtypes.bfloat16
    x = np.asarray(inputs["x"], dtype=np.float32)
    adj = np.asarray(inputs["adj"], dtype=np.float32)
    W = np.asarray(inputs["W"], dtype=np.float32)
    b = np.asarray(inputs["b"], dtype=np.float32)

    x_bf = np.ascontiguousarray(x.astype(bf))
    wt_bf = np.ascontiguousarray(W.T.astype(bf))
    adjt_bf = adj.astype(bf).T  # [K, rows] view
    b_tiled = np.ascontiguousarray(b.reshape(JT, P).T)  # [128, 4]

    in_maps = []
    for c in range(NCORES):
        in_maps.append(
            {
                "x": x_bf,
                "adjt": np.ascontiguousarray(adjt_bf[:, c * M : (c + 1) * M]),
                "wt": wt_bf,
                "b": b_tiled,
            }
        )
    return in_maps


def _run(in_maps, **kwargs):
    return run_bass_kernel_spmd(
        _get_nc(), in_maps, core_ids=list(range(NCORES)), **kwargs
    )


def _assemble(results):
    out = np.empty((N_NODES, DIM), dtype=np.float32)
    for c in range(NCORES):
        out[c * M : (c + 1) * M, :] = results[c]["outt"].T
    return out


def kernel(**inputs):
    res = _run(_prepare(inputs))
    return _assemble(res.results)
